# revision 36
# baseline (speedup 1.0000x reference)
"""Trainium2 Bass kernel for 16-head MHA with RoPE (B=1, S=4096, D=1024).

Sharding: tensor-parallel over heads — 2 heads per core on 8 cores.
Key structure (all matmuls bf16, fp32 PSUM accumulation):
  * Score matmuls issued in (h0, h1)-adjacent pairs so the two K=64
    contractions run CONCURRENTLY in the PE's 64x128 row-tiled mode
    (tile_position auto-derives from base_partition 0/64).
  * exp() split across TWO engines: ScalarE runs the accurate ACT
    spline; VectorE produces bf16 exp approximations with a single
    2-op tensor_scalar via the exponent bit-trick: wk is host-prescaled
    by 16*log2(e) so the score PSUM value is u = 128*log2(exp-weight);
    round(u + C2) IS the bf16 bit pattern of the weight (the int16
    convert rounds to nearest). ~12/32 of the exp groups take the
    +-2.5% sawtooth; softmax ratio-cancellation keeps the output error
    within budget.
  * qtile-boundary de-stall: den + ctx evacuations split across
    ScalarE and VectorE in parallel so the next qtile's first ctx
    accumulation (start=True on the same PSUM banks) waits <1.5us —
    short enough that the PE's HAM clock gate stays warm.
  * 1/den via exp(-ln(den)) on ScalarE (one ACT table set covers
    Ln+Exp+Copy; vector.reciprocal costs 3.3us/qtile).
  * hidT arrives as 8 chunk-tiles so the DMA-paced Q projection starts
    on the first chunk instead of waiting for the full 8MB.
Host sums the 8 fp16 partials in fp32.
"""

import functools

import numpy as np
import ml_dtypes

import concourse.bass as bass
import concourse.tile as tile
import concourse.mybir as mybir
from concourse.bass_utils import run_bass_kernel_spmd

BF16 = mybir.dt.bfloat16
I16 = mybir.dt.int16
F16 = mybir.dt.float16
F32 = mybir.dt.float32
bf16 = ml_dtypes.bfloat16

S = 4096      # sequence length
D = 1024      # model dim
HD = 64       # head dim
C = 128       # channels per core (2 heads)
NDC = 8       # contraction chunks of 128 over D
NKC = 32      # key chunks of 128 over S
NPR = 16      # key chunk-pairs of 256 over S
NQT = 8       # query tiles of 512
QT = 512
VB = 66       # v4 block stride per (chunk, head): 64 V cols + f col + pad

# wk is host-prescaled by PRE so score PSUM u = 128*log2(exp-weight)
PRE = 0.125 * np.log2(np.e) * 128.0          # 23.083120654223414
ACT_SCALE = 0.125 / PRE                      # exact exp path: exp(u*ACT_SCALE)
# bf16 bit-trick: i16 = round(u + C2): 16256 centers bf16 exponent bias;
# -128*E[log2((1+f)/2^f)] removes the Schraudolph sawtooth's mean bias.
C2_TRICK = 16256.0 - 128.0 * 0.05730355757  # = 16248.665
NDVE = 12     # of the 32 (pair, head) exp groups per qtile, how many on DVE


def _dve_groups(n):
    s = set()
    acc = 0
    for g in range(32):
        acc2 = ((g + 1) * n) // 32
        if acc2 > acc:
            s.add(g)
        acc = acc2
    return s


DVE_GROUPS = _dve_groups(NDVE)
# group 0 is the first exp after the qtile boundary: keep it off the ACT
# FIFO (which also carries the den/ctx evacuations there)
DVE_GROUPS = (DVE_GROUPS - {2}) | {0}

_NO_SPLIT = (
    mybir.InstEventSemaphore,
    mybir.InstUnconditionalBranch,
)


def _split_multi_waits(nc: bass.Bass) -> None:
    """Hoist extra sem waits onto standalone EventSemaphore carriers.

    This walrus build only supports one sync-wait command per engine
    instruction ("Too many sync wait commands" in setupSyncWait), so any
    instruction Tile scheduled with >1 wait gets all but its last wait moved
    to dedicated InstEventSemaphore instructions placed immediately before it
    in the same engine stream (sequencer blocks on them in program order —
    semantically identical).
    """
    n = 0
    for fn in nc.m.functions:
        for blk in fn.blocks:
            out = []
            for inst in blk.instructions:
                si = inst.sync_info
                if (
                    si is not None
                    and si.on_wait
                    and len(si.on_wait) > 1
                    and not isinstance(inst, _NO_SPLIT)
                    and inst.engine != mybir.EngineType.Unassigned
                ):
                    waits = list(si.on_wait)
                    for w in waits[:-1]:
                        ev = mybir.InstEventSemaphore(name=f"ant_waitsplit_{n}")
                        n += 1
                        ev.engine = inst.engine
                        ev.sync_info = mybir.SyncInfo(on_wait=[w], on_update=[])
                        nc.register_instruction(ev)
                        out.append(ev)
                    si.on_wait = [waits[-1]]
                    inst.sync_info = si
                out.append(inst)
            blk.instructions[:] = out


def build_program() -> bass.Bass:
    nc = bass.Bass()
    hidT_d = nc.declare_dram_parameter("hidT", [D, S], BF16, isOutput=False)
    wq_d = nc.declare_dram_parameter("wq", [128, D], BF16, isOutput=False)
    wk_d = nc.declare_dram_parameter("wk", [128, D], BF16, isOutput=False)
    wv_d = nc.declare_dram_parameter("wv", [128, D], BF16, isOutput=False)
    wo_d = nc.declare_dram_parameter("wo", [128, D], BF16, isOutput=False)
    cos_d = nc.declare_dram_parameter("cosf", [128, S], BF16, isOutput=False)
    sin_d = nc.declare_dram_parameter("sinf", [128, S], BF16, isOutput=False)
    mask_d = nc.declare_dram_parameter("maskadd", [128, NKC], F32, isOutput=False)
    out_d = nc.declare_dram_parameter("outp", [S, D], F16, isOutput=True)

    Exp = mybir.ActivationFunctionType.Exp
    Ln = mybir.ActivationFunctionType.Ln
    mult = mybir.AluOpType.mult
    add = mybir.AluOpType.add
    amax = mybir.AluOpType.max

    with tile.TileContext(nc) as tc:
        with (
            tc.tile_pool(name="const", bufs=1) as const,
            tc.tile_pool(name="ppool", bufs=13) as ppool,
            tc.tile_pool(name="sgps", bufs=3, space="PSUM") as sgps,
            tc.tile_pool(name="ctxps", bufs=1, space="PSUM") as ctxps,
            tc.tile_pool(name="rpool", bufs=2) as rpool,
            tc.tile_pool(name="spool", bufs=1) as spool,
            tc.tile_pool(name="opool", bufs=2) as opool,
        ):
            # ---- persistent SBUF tiles -------------------------------------
            wq_sb = const.tile([128, D], BF16, tag="wq")
            wk_sb = const.tile([128, D], BF16, tag="wk")
            wv_sb = const.tile([128, D], BF16, tag="wv")
            wo_sb = const.tile([128, D], BF16, tag="wo")
            mask_sb = const.tile([128, NKC], F32, tag="mask")
            f_f32 = const.tile([128, NKC], F32, tag="ff32")
            cos_sb = const.tile([128, S], BF16, tag="cosf")
            sin_sb = const.tile([128, S], BF16, tag="sinf")
            qT_bf = const.tile([128, S], BF16, tag="qTbf")
            kT_bf = const.tile([128, S], BF16, tag="kTbf")
            qsw = const.tile([128, S], BF16, tag="qsw")
            v4 = const.tile([128, NKC * 2 * VB], BF16, tag="v4")
            ctxn = const.tile([128, S], BF16, tag="ctxn")
            # one tile per 128-row chunk so Tile tracks DMA completion at
            # chunk granularity (a single big tile stalls the first
            # projection matmul until ALL hidT DMAs land)
            hidT_sb = [
                const.tile([128, S], BF16, tag=f"hidT{dc}", name=f"hidT{dc}")
                for dc in range(NDC)
            ]
            vT_bf = const.tile([128, S], BF16, tag="vTbf")
            v_nat = const.tile([128, S], BF16, tag="vnat")

            # hidT alternates between the two hwdge queues; the scalar queue
            # additionally carries everything needed later, ordered by first
            # use.
            nc.sync.dma_start(out=wk_sb[:], in_=wk_d[:])
            nc.scalar.dma_start(out=mask_sb[:], in_=mask_d[:])
            for dc in range(NDC):
                eng = nc.sync if dc % 2 == 0 else nc.scalar
                eng.dma_start(
                    out=hidT_sb[dc][:],
                    in_=hidT_d[dc * 128 : (dc + 1) * 128, :],
                )
            nc.scalar.dma_start(out=wq_sb[:], in_=wq_d[:])
            nc.sync.dma_start(out=cos_sb[:], in_=cos_d[:])
            nc.sync.dma_start(out=sin_sb[:], in_=sin_d[:])
            nc.scalar.dma_start(out=wv_sb[:], in_=wv_d[:])
            nc.scalar.dma_start(out=wo_sb[:], in_=wo_d[:])
            # f[k] = exp(mask_add[k]) — also warms the ACT exp table early
            nc.scalar.activation(f_f32[:], mask_sb[:], Exp)

            # ---- PSUM layout ----------------------------------------------
            # full-bank tiles: rows 0-64 hold the ctx accumulation; during
            # qtile 0's lead-in the V projection strips borrow them whole.
            ctxA = ctxps.tile([128, QT], F32, tag="ctxA")
            ctxB = ctxps.tile([128, QT], F32, tag="ctxB")
            ctx_banks = (ctxA, ctxB)

            ones_bf = const.tile([1, 64], BF16, tag="onesbf")
            nc.vector.memset(ones_bf[:], 1.0)

            # pre-fill both dsb ring buffers so the batched 1/den's
            # untouched rows (1..31) stay finite
            for _ in range(2):
                dpre = rpool.tile([33, QT], F32, tag="dsb", name="dpre")
                nc.vector.memset(dpre[:], 1.0)

            # ---- phase 1: projections through the score ring ---------------
            def mm_proj(t, jslot, w_sb, strip, dc):
                nc.tensor.matmul(
                    t[:, jslot * QT : (jslot + 1) * QT],
                    lhsT=w_sb[:, dc * 128 : (dc + 1) * 128],
                    rhs=hidT_sb[dc][:, strip * QT : (strip + 1) * QT],
                    start=(dc == 0),
                    stop=(dc == NDC - 1),
                )

            def proj_ring(w_sb, dst, dma_paced=False, per_pair=None,
                          skip_tail=False):
                """8 strips of 512 via ring tiles of 2+2+2 strips + tail."""
                t0 = sgps.tile([128, 2 * QT], F32, tag="sg", name="pj0")
                t1 = sgps.tile([128, 2 * QT], F32, tag="sg", name="pj1")
                t2 = sgps.tile([128, 2 * QT], F32, tag="sg", name="pj2")
                tiles = (t0, t1, t2)
                if dma_paced:
                    # strips 0-5 consume hidT chunks as they arrive; strips
                    # 6-7 re-read SBUF-resident chunks in a second pass
                    for dc in range(NDC):
                        for strip in range(6):
                            mm_proj(tiles[strip // 2], strip % 2, w_sb, strip, dc)
                else:
                    for strip in range(6):
                        for dc in range(NDC):
                            mm_proj(tiles[strip // 2], strip % 2, w_sb, strip, dc)
                for p in range(3):
                    nc.scalar.copy(
                        dst[:, p * 2 * QT : (p + 1) * 2 * QT], tiles[p][:, :]
                    )
                    if per_pair is not None:
                        per_pair(p)
                if skip_tail:
                    return
                t3 = sgps.tile([128, 2 * QT], F32, tag="sg", name="pj3")
                for strip in range(6, 8):
                    for dc in range(NDC):
                        mm_proj(t3, strip - 6, w_sb, strip, dc)
                nc.scalar.copy(dst[:, 6 * QT : 8 * QT], t3[:, :])
                if per_pair is not None:
                    per_pair(3)

            def rope(x_bf, s0, s1):
                # channel rows per head h: [h*64, h*64+32) = evens ("a"),
                # [h*64+32, h*64+64) = odds ("b");
                # out = x * cos_full + swap(x) * sin_signed
                sc = slice(s0, s1)
                for h in range(2):
                    a = slice(h * 64, h * 64 + 32)
                    b = slice(h * 64 + 32, h * 64 + 64)
                    nc.vector.tensor_copy(qsw[a, sc], x_bf[b, sc])
                    nc.vector.tensor_copy(qsw[b, sc], x_bf[a, sc])
                nc.vector.tensor_tensor(x_bf[:, sc], x_bf[:, sc], cos_sb[:, sc], op=mult)
                nc.vector.tensor_tensor(qsw[:, sc], qsw[:, sc], sin_sb[:, sc], op=mult)
                nc.vector.tensor_tensor(x_bf[:, sc], x_bf[:, sc], qsw[:, sc], op=add)

            v4r = v4[:].rearrange("p (kc h c) -> p kc h c", kc=NKC, h=2)
            vnr = v_nat[:].rearrange("p (kc h c) -> p kc h c", kc=NKC, h=2)

            def restride(kc0, kc1):
                # fused f-scale + restride of v_nat chunks into v4 blocks
                for kc in range(kc0, kc1):
                    nc.vector.tensor_scalar(
                        v4r[:, kc : kc + 1, :, 0:64],
                        vnr[:, kc : kc + 1, :, :],
                        f_f32[:, kc : kc + 1],
                        None,
                        op0=mult,
                    )

            # K FIRST, DMA-paced, with per-pair rope chasing the casts
            # (scores need the full kT but only qT strip 0, so K owns the
            # DMA-chase; Q shrinks to strips 0-1 here and pairs 1-3 are
            # deferred into qtile 0/1's block loops).
            proj_ring(
                wk_sb,
                kT_bf,
                dma_paced=True,
                per_pair=lambda p: rope(kT_bf, p * 2 * QT, (p + 1) * 2 * QT),
            )

            def emit_qpair(p):
                t = sgps.tile([128, 2 * QT], F32, tag="sg", name=f"qp{p}")
                for strip in (2 * p, 2 * p + 1):
                    for dc in range(NDC):
                        mm_proj(t, strip - 2 * p, wq_sb, strip, dc)
                nc.scalar.copy(qT_bf[:, 2 * p * QT : (p + 1) * 2 * QT], t[:, :])
                rope(qT_bf, p * 2 * QT, (p + 1) * 2 * QT)

            emit_qpair(0)

            # f columns of v4 (written before any restride touches v4)
            for h in range(2):
                nc.scalar.copy(
                    v4r[:, :, h : h + 1, 64:65],
                    f_f32[:].unsqueeze(-1).unsqueeze(-1),
                )

            def emit_vstrip(strip):
                bank = ctx_banks[strip % 2]
                for dc in range(NDC):
                    mm_proj(bank, 0, wv_sb, strip, dc)
                nc.scalar.copy(
                    vT_bf[:, strip * QT : (strip + 1) * QT], bank[:, :]
                )

            def emit_vquarter(q):
                # transpose strips 2q,2q+1 into v_nat chunks 8q..8q+7, then
                # f-scale + restride them into v4
                nc.sync.dma_start_transpose(
                    out=v_nat[:].rearrange("p (kc c) -> p kc c", kc=NKC)[
                        :, 8 * q : 8 * (q + 1), :
                    ],
                    in_=vT_bf[:, q * 2 * QT : (q + 1) * 2 * QT],
                )
                restride(8 * q, 8 * (q + 1))

            # ---- phase 2: attention + fused out-projection -----------------
            def emit_scores(qt, a):
                """Score block for pair a, both heads: 4 MMs issued so the
                two heads' K=64 matmuls run concurrently in the 64x128
                row-tiled PE mode. Returns (PtA, PtB)."""
                qc = slice(qt * QT, (qt + 1) * QT)
                sgh = []
                for h in range(2):
                    sgh.append(
                        sgps.tile([128, 2 * QT], F32, tag="sg", name=f"sg{h}")
                    )
                for j in range(2):
                    c = 2 * a + j
                    for h in range(2):
                        hr = slice(h * 64, (h + 1) * 64)
                        nc.tensor.matmul(
                            sgh[h][:, j * QT : (j + 1) * QT],
                            lhsT=kT_bf[hr, c * 128 : (c + 1) * 128],
                            rhs=qT_bf[hr, qc],
                            start=True,
                            stop=True,
                        )
                pts = []
                for h in range(2):
                    pt = ppool.tile([128, 2 * QT], BF16, tag="pt", name=f"pt{h}")
                    if (a * 2 + h) in DVE_GROUPS:
                        nc.vector.tensor_scalar(
                            pt[:].bitcast(I16),
                            sgh[h][:, :],
                            C2_TRICK,
                            0.0,
                            op0=add,
                            op1=amax,
                        )
                    else:
                        nc.scalar.activation(
                            pt[:], sgh[h][:, :], Exp, scale=ACT_SCALE
                        )
                    pts.append(pt)
                return pts

            def emit_ctx(a, pts):
                for h in range(2):
                    pt = pts[h]
                    for j in range(2):
                        c = 2 * a + j
                        vcol = (c * 2 + h) * VB
                        nc.tensor.matmul(
                            ctx_banks[h][0:65, :],
                            lhsT=v4[:, vcol : vcol + 65],
                            rhs=pt[:, j * QT : (j + 1) * QT],
                            start=(c == 0),
                            stop=(c == NKC - 1),
                        )

            def emit_recips(qt):
                # den + ctx evacuated right after the final ctx accumulation,
                # SPLIT across ScalarE (bank A) and VectorE (bank B) so the
                # serial chain before the next qtile's first ctx (start=True
                # on these banks) stays ~1.4us — short enough that the PE
                # never idles into a HAM re-throttle. The reciprocal chain
                # itself is deferred (emit_recips2 at block 1 of the next
                # qtile) so it does not head-of-line-block the next qtile's
                # exp stream in the ACT/DVE FIFOs.
                dsb = rpool.tile([33, QT], F32, tag="dsb")
                ctxc = rpool.tile([128, QT], BF16, tag="ctxc")
                nc.scalar.copy(dsb[0:1, :], ctxA[64:65, :])
                nc.vector.tensor_copy(dsb[32:33, :], ctxB[64:65, :])
                nc.scalar.copy(ctxc[0:64, :], ctxA[0:64, :])
                nc.vector.tensor_copy(ctxc[64:128, :], ctxB[0:64, :])
                return dsb, ctxc

            def emit_recips2(dsb):
                # 1/den = exp(-ln(den)) on ScalarE; one table set covers
                # Ln+Exp+Copy (vector.reciprocal would cost 3.3us).
                tln = spool.tile([33, QT], F32, tag="tln")
                nc.scalar.activation(tln[:, :], dsb[:, :], Ln)
                rcp = rpool.tile([33, QT], F32, tag="rcp")
                nc.scalar.activation(rcp[:, :], tln[:, :], Exp, scale=-1.0)
                rsb = rpool.tile([1, 2 * QT], BF16, tag="rsb")
                nc.vector.tensor_copy(rsb[0:1, 0:QT], rcp[0:1, :])
                nc.vector.tensor_copy(rsb[0:1, QT : 2 * QT], rcp[32:33, :])
                return rsb

            def emit_normalize(qt, rsb, ctxc):
                # broadcast recips across partitions via rank-1 PE matmuls
                # into a borrowed score-ring bank, then fused normalize
                # into ctxn (bf16 tensor_tensor at 2x).
                qc = slice(qt * QT, (qt + 1) * QT)
                bt = sgps.tile([128, 2 * QT], F32, tag="sg")
                nc.tensor.matmul(
                    bt[0:64, 0:QT],
                    lhsT=ones_bf[:],
                    rhs=rsb[0:1, 0:QT],
                    start=True,
                    stop=True,
                )
                nc.tensor.matmul(
                    bt[64:128, 0:QT],
                    lhsT=ones_bf[:],
                    rhs=rsb[0:1, QT : 2 * QT],
                    start=True,
                    stop=True,
                    tile_position=(0, 64),
                )
                recb = rpool.tile([128, QT], BF16, tag="recb")
                nc.vector.tensor_copy(recb[:], bt[:, 0:QT])
                nc.vector.tensor_tensor(
                    ctxn[0:64, qc], ctxc[0:64, :], recb[0:64, :], op=mult
                )
                nc.vector.tensor_tensor(
                    ctxn[64:128, qc], ctxc[64:128, :], recb[64:128, :], op=mult
                )

            def emit_outproj(qt, j, cast_eng=None):
                # out rows [qt*512 + j*128, +128) = ctxn_chunk^T @ wo;
                # output DMAs alternate between the two hwdge queues
                ot = sgps.tile([128, 2 * QT], F32, tag="sg")
                col = qt * QT + j * 128
                for half in range(2):
                    nc.tensor.matmul(
                        ot[:, half * QT : (half + 1) * QT],
                        lhsT=ctxn[:, col : col + 128],
                        rhs=wo_sb[:, half * QT : (half + 1) * QT],
                        start=True,
                        stop=True,
                    )
                osb = opool.tile([128, D], F16, tag="osb")
                (cast_eng or nc.vector.tensor_copy)(osb[:], ot[:, 0:D])
                # all output DMAs issue from the Sync engine: a DMA_DIRECT2D
                # costs ~640ns of issue time on its engine's instruction
                # stream, and ScalarE is the block-pacing engine (Sync is
                # ~8% busy)
                nc.sync.dma_start(out=out_d[col : col + 128, :], in_=osb[:])

            # qtile 0 interleaves the V projection into its first 4 score
            # blocks (2 strips each through the still-idle ctx banks) and
            # defers ctx by 5 blocks until the first v4 quarters land; later
            # qtiles run with lag 1. outproj j=3 of qtile t is emitted just
            # AFTER the (t+1 -> t+2) qtile boundary so the PE has work while
            # the exp backlog at the boundary drains.
            norm_prev = None
            for qt in range(NQT):
                lag = 5 if qt == 0 else 1
                pend = {}
                for a in range(NPR):
                    pend[a] = emit_scores(qt, a)
                    if qt == 0 and a < 4:
                        emit_vstrip(2 * a)
                        emit_vstrip(2 * a + 1)
                        emit_vquarter(a)
                    if qt == 0 and a in (8, 12):
                        emit_qpair({8: 1, 12: 2}[a])
                    if qt == 1 and a == 6:
                        emit_qpair(3)
                    if a == 1 and qt > 0:
                        rsb_prev = emit_recips2(norm_prev[0])
                    if a == 2 and qt > 0:
                        emit_normalize(qt - 1, rsb_prev, norm_prev[1])
                    if qt > 0 and a in (4, 7, 10, 13):
                        emit_outproj(qt - 1, {4: 0, 7: 1, 10: 2, 13: 3}[a])
                    if a >= lag:
                        emit_ctx(a - lag, pend.pop(a - lag))
                for a in range(NPR - lag, NPR):
                    emit_ctx(a, pend.pop(a))
                norm_prev = emit_recips(qt)
            emit_normalize(NQT - 1, emit_recips2(norm_prev[0]), norm_prev[1])
            for j in range(4):
                # alternate the final copies between DVE and ScalarE to
                # shorten the serial tail
                emit_outproj(
                    NQT - 1, j,
                    cast_eng=nc.scalar.copy if j % 2 else nc.vector.tensor_copy,
                )

    _split_multi_waits(nc)
    return nc


@functools.cache
def _cached_program() -> bass.Bass:
    return build_program()


def _prep_inputs(hidden_states, freqs_cis, attention_mask, wq, wk, wv, wo):
    hid = np.asarray(hidden_states, np.float32).reshape(S, D)
    hidT = np.ascontiguousarray(hid.T).astype(bf16)

    # within-head channel permutation: evens then odds (rope pairs 32 apart)
    perm1 = np.concatenate([np.arange(0, HD, 2), np.arange(1, HD, 2)])
    perm = np.concatenate([perm1, perm1 + HD])  # for the 2 heads of a core

    fc = np.asarray(freqs_cis, np.float32)
    cosT = np.ascontiguousarray(fc[:, :, 0].T)  # [32, S]
    sinT = np.ascontiguousarray(fc[:, :, 1].T)
    cosf = np.concatenate([cosT, cosT, cosT, cosT], 0).astype(bf16)
    sinf = np.concatenate([-sinT, sinT, -sinT, sinT], 0).astype(bf16)

    mask_add = (1.0 - np.asarray(attention_mask, np.float32).reshape(S)) * -10000.0
    maskadd = np.ascontiguousarray(mask_add.reshape(NKC, 128).T).astype(np.float32)

    def wlayout(w):  # [1024, 128] -> [128 partitions, chunk-major 1024]
        w = np.ascontiguousarray(w)
        return np.ascontiguousarray(
            w.reshape(NDC, 128, 128).transpose(1, 0, 2).reshape(128, D)
        ).astype(bf16)

    in_maps = []
    for core in range(8):
        cols = slice(core * 128, (core + 1) * 128)
        in_maps.append(
            {
                "hidT": hidT,
                "wq": wlayout(np.asarray(wq, np.float32)[:, cols][:, perm]),
                "wk": wlayout(
                    np.asarray(wk, np.float32)[:, cols][:, perm] * PRE
                ),
                "wv": wlayout(np.asarray(wv, np.float32)[:, cols]),
                "wo": np.ascontiguousarray(np.asarray(wo, np.float32)[cols, :]).astype(bf16),
                "cosf": cosf,
                "sinf": sinf,
                "maskadd": maskadd,
            }
        )
    return in_maps


def run_sharded(in_maps, **kwargs):
    nc = _cached_program()
    return run_bass_kernel_spmd(nc, in_maps, list(range(8)), **kwargs)


def kernel(hidden_states, freqs_cis, attention_mask, wq, wk, wv, wo):
    in_maps = _prep_inputs(
        hidden_states, freqs_cis, attention_mask, wq, wk, wv, wo
    )
    res = run_sharded(in_maps).results
    out = np.zeros((S, D), np.float32)
    for r in res:
        out += np.asarray(r["outp"], np.float32)
    return out.reshape(1, S, D)


if __name__ == "__main__":
    import reference

    inputs = reference.setup_inputs()
    inputs = {k: np.asarray(v) for k, v in inputs.items()}
    expected = np.asarray(reference.reference(**inputs))
    actual = kernel(**inputs)
    err = np.abs(actual - expected).max() / np.abs(expected).max()
    print("Relative error:", err)
